# revision 1
# baseline (speedup 1.0000x reference)
"""Trainium2 Bass kernel for nn_CausalSelfAttention_78331613544603.

Tensor-parallel over heads across 8 NeuronCores (Megatron-style):
each core computes QKV for its 2 heads, runs causal attention for its
(batch, head) pairs, projects with its w_proj column-slice, and the
partial outputs are combined with chunked ReduceScatter collectives.
The host shards the weights and gathers the output shards.

Self-contained: only needs numpy + the concourse toolchain staged at
/opt/trn_rl_repo (also importable via the environment's PYTHONPATH).
"""

import math
import sys

import numpy as np

try:
    import concourse.bass as bass
except ImportError:
    sys.path.insert(0, "/opt/trn_rl_repo")
    import concourse.bass as bass

import concourse.mybir as mybir
import concourse.tile as tile
from concourse import bacc, bass_utils

F32 = mybir.dt.float32
F32R = mybir.dt.float32r
BF16 = mybir.dt.bfloat16

N_CORES = 8
HEADS = 16
HPC = HEADS // N_CORES  # heads per core = 2
HD = 256  # head dim
KV_CHANNELS = 128
NEG = -1.0e30


class Cfg:
    def __init__(self, seq=2048, e=4096, out=2048):
        self.seq = seq  # sequence length
        self.batch = 2
        self.e = e  # input embedding dim (2*HIDDEN)
        self.out = out  # output dim (HIDDEN)
        self.ech = e // 128  # contraction chunks
        self.tok = seq * self.batch  # total tokens (batch-major)
        self.ntb = self.tok // 256  # qkv token blocks
        self.supers = seq // 256  # q super-tiles per (b,h)
        self.f_qk = HPC * HD * 2  # 1024 local q+k features
        self.f_v = HPC * HD  # 512 local v features
        self.nstg = self.tok // 512  # reduce-scatter chunks


def build_kernel(cfg: Cfg, debug_dumps=False):
    nc = bacc.Bacc("TRN2", target_bir_lowering=False, debug=False,
                   num_devices=N_CORES)

    ECH = cfg.ech
    SEQ = cfg.seq
    TOK = cfg.tok
    OUT = cfg.out

    # ---- kernel I/O ----
    hs5 = nc.dram_tensor("hs5", [cfg.ntb, ECH, 128, 256], F32R,
                         kind="ExternalInput")
    wqk = nc.dram_tensor("wqk", [ECH, 128, cfg.f_qk], F32R,
                         kind="ExternalInput")
    wv = nc.dram_tensor("wv", [ECH, 128, cfg.f_v], F32R, kind="ExternalInput")
    wp = nc.dram_tensor("wp", [4, 128, OUT], F32R, kind="ExternalInput")
    maskm = nc.dram_tensor("maskm", [128, 1024], F32, kind="ExternalInput")
    ident = nc.dram_tensor("ident", [128, 128], BF16, kind="ExternalInput")
    out_ext = nc.dram_tensor("out_ext", [cfg.nstg, 512 // N_CORES, OUT], F32,
                             kind="ExternalOutput")
    if debug_dumps:
        qk_dump = nc.dram_tensor("qk_dump", [8, 128, TOK], F32,
                                 kind="ExternalOutput")
        v_dump = nc.dram_tensor("v_dump", [128, TOK // 128, cfg.f_v], F32,
                                kind="ExternalOutput")
        yt_dump = nc.dram_tensor("yt_dump", [128, 4, TOK], F32,
                                 kind="ExternalOutput")
        part_dump = nc.dram_tensor("part_dump", [TOK, OUT], F32,
                                   kind="ExternalOutput")

    with tile.TileContext(nc) as tc:
        with (
            tc.tile_pool(name="const", bufs=1) as constp,
            tc.tile_pool(name="resident", bufs=1) as resp,
            tc.tile_pool(name="dram", bufs=1, space="DRAM") as dramp,
        ):
            mask_sb = constp.tile([128, 1024], F32, name="mask_sb")
            nc.sync.dma_start(mask_sb[:], maskm.ap())
            ident_sb = constp.tile([128, 128], BF16, name="ident_sb")
            nc.sync.dma_start(ident_sb[:], ident.ap())

            # v for all tokens stays resident through attention
            v_all = resp.tile([128, TOK // 128, cfg.f_v], BF16, name="v_all")

            qk_spill = dramp.tile([8, 128, TOK], F32, name="qk_spill")
            partial_c = [dramp.tile([512, OUT], F32, name=f"partial{c}",
                                    tag=f"partial{c}")
                         for c in range(cfg.nstg)]
            rs_out_c = [dramp.tile([512 // N_CORES, OUT], F32,
                                   name=f"rs_out{c}", tag=f"rs_out{c}")
                        for c in range(cfg.nstg)]

            # ================= phase 1: QK projection =================
            with (
                nc.named_scope("qk_proj"),
                tc.tile_pool(name="p1", bufs=1) as p1,
                tc.tile_pool(name="p1hs", bufs=2) as p1hs,
                tc.tile_pool(name="p1st", bufs=4) as p1st,
                tc.tile_pool(name="ps1", bufs=1, space="PSUM") as ps1,
            ):
                wqk_sb = {}
                for j in range(4):
                    for eh in range(2):
                        wqk_sb[(j, eh)] = p1.tile(
                            [128, ECH // 2, 256], F32R,
                            name=f"wqk_sb{j}_{eh}", tag=f"wqk_sb{j}_{eh}")

                def load_w(j, eh):
                    nc.sync.dma_start(
                        wqk_sb[(j, eh)][:],
                        wqk.ap()[eh * (ECH // 2):(eh + 1) * (ECH // 2), :,
                                 j * 256:(j + 1) * 256]
                        .rearrange("ec p f -> p ec f"),
                    )

                for tb in range(cfg.ntb):
                    pst = [
                        ps1.tile([128, 256], F32, name=f"ps_qk{fc}",
                                 tag=f"ps_qk{fc}")
                        for fc in range(8)
                    ]
                    for eh in range(2):
                        hs_t = p1hs.tile([128, ECH // 2, 256], F32R,
                                         name="hs_t", tag="hs_t")
                        nc.sync.dma_start(
                            hs_t[:],
                            hs5.ap()[tb, eh * (ECH // 2):(eh + 1) * (ECH // 2)]
                            .rearrange("ec p t -> p ec t"),
                        )
                        for j in range(4):
                            if tb == 0:
                                load_w(j, eh)
                            for el in range(ECH // 2):
                                ec = eh * (ECH // 2) + el
                                for half in range(2):
                                    fc = 2 * j + half
                                    nc.tensor.matmul(
                                        pst[fc][:],
                                        wqk_sb[(j, eh)][:, el,
                                                        half * 128:
                                                        (half + 1) * 128],
                                        hs_t[:, el, :],
                                        start=(ec == 0),
                                        stop=(ec == ECH - 1),
                                    )
                    for fc in range(8):
                            stg = p1st.tile([128, 256], F32, name="qkstg",
                                            tag="qkstg")
                            nc.vector.tensor_copy(stg[:], pst[fc][:])
                            nc.sync.dma_start(
                                qk_spill[fc, :, tb * 256:(tb + 1) * 256],
                                stg[:])
                            if debug_dumps:
                                nc.sync.dma_start(
                                    qk_dump.ap()[fc, :,
                                                 tb * 256:(tb + 1) * 256],
                                    stg[:])

            # ================= phase 2: V projection =================
            with (
                nc.named_scope("v_proj"),
                tc.tile_pool(name="p2", bufs=1) as p2,
                tc.tile_pool(name="p2hs", bufs=2) as p2hs,
                tc.tile_pool(name="ps2", bufs=3, space="PSUM") as ps2,
            ):
                wv_sb = p2.tile([128, ECH, cfg.f_v], F32R, name="wv_sb")
                nc.gpsimd.dma_start(wv_sb[:],
                                    wv.ap().rearrange("ec p f -> p ec f"))
                for tb in range(cfg.ntb):
                    for eh in range(2):
                        hs_t2 = p2hs.tile([128, ECH // 2, 256], F32R,
                                          name="hs_t2", tag="hs_t2")
                        nc.sync.dma_start(
                            hs_t2[:],
                            hs5.ap()[tb, eh * (ECH // 2):(eh + 1) * (ECH // 2)]
                            .rearrange("ec p t -> p ec t"),
                        )
                        if eh == 0:
                            ps_v = [
                                ps2.tile([128, cfg.f_v], F32, name="ps_v",
                                         tag="ps_v")
                                for _ in range(2)
                            ]
                        for tc_i in range(2):
                            for el in range(ECH // 2):
                                ec = eh * (ECH // 2) + el
                                nc.tensor.matmul(
                                    ps_v[tc_i][:],
                                    hs_t2[:, el, tc_i * 128:(tc_i + 1) * 128],
                                    wv_sb[:, ec, :],
                                    start=(ec == 0),
                                    stop=(ec == ECH - 1),
                                )
                    for tc_i in range(2):
                        g = tb * 2 + tc_i
                        nc.vector.tensor_copy(v_all[:, g, :], ps_v[tc_i][:])
                if debug_dumps:
                    vstg = p2.tile([128, TOK // 128, cfg.f_v], F32,
                                   name="vstg")
                    nc.vector.tensor_copy(vstg[:], v_all[:])
                    nc.sync.dma_start(v_dump.ap(), vstg[:])

            # y^T per batch, alive from attention through the projection
            ytp_cm = tc.tile_pool(name="ytp", bufs=1)
            ytp = ytp_cm.__enter__()
            nstg_b = cfg.nstg // cfg.batch
            yt_t = {}
            for b in range(cfg.batch):
                for sl in range(nstg_b):
                    yt_t[(b, sl)] = ytp.tile(
                        [128, 4, 512], F32R, name=f"yt_{b}_{sl}",
                        tag=f"yt_{b}_{sl}")

            # ====== phases 3+4: attention software-pipelined with =========
            # ====== per-batch output projection + ReduceScatter    =========
            with (
                nc.named_scope("attn_proj"),
                tc.tile_pool(name="qkp", bufs=2) as qkp,
                tc.tile_pool(name="qsp", bufs=2) as qsp,
                tc.tile_pool(name="strips", bufs=2) as strips,
                tc.tile_pool(name="ptp", bufs=2) as ptp,
                tc.tile_pool(name="statp", bufs=4) as statp,
                tc.tile_pool(name="wpp", bufs=2) as wpp,
                tc.tile_pool(name="pstrips", bufs=2) as pstrips,
                tc.tile_pool(name="p4st", bufs=2) as p4st,
                tc.tile_pool(name="ps3", bufs=2, space="PSUM") as ps3,
                tc.tile_pool(name="ps3b", bufs=1, space="PSUM") as ps3b,
                tc.tile_pool(name="ps3o", bufs=2, space="PSUM") as ps3o,
            ):
                n_ob = OUT // 512
                kp_tiles = {}

                def produce(b, h, u):
                    """QK for one q-super: S blocks -> masked strips."""
                    if u == 0:
                        kp = qkp.tile([128, 2, SEQ], F32, name="kp", tag="kp")
                        for dc in range(2):
                            nc.sync.dma_start(
                                kp[:, dc, :],
                                qk_spill[4 + 2 * h + dc, :,
                                         b * SEQ:(b + 1) * SEQ])
                        kp_tiles[(b, h)] = kp
                    kp = kp_tiles[(b, h)]
                    qs = qsp.tile([128, 2, 256], F32, name="qs", tag="qs")
                    for dc in range(2):
                        nc.sync.dma_start(
                            qs[:, dc, :],
                            qk_spill[2 * h + dc, :,
                                     b * SEQ + u * 256:b * SEQ + (u + 1) * 256])
                    nb = (u + 2) // 2
                    # PV only reads cols [0, (2u+2)*128); trim the last
                    # block to 256 wide when nb*512 overshoots by 256
                    rem = nb * 512 - (2 * u + 2) * 128
                    strip = [
                        strips.tile([128, SEQ], F32, name=f"strip{qt}",
                                    tag=f"strip{qt}")
                        for qt in range(2)
                    ]
                    for qt in range(2):
                        i = 2 * u + qt
                        d_jb = i // 4
                        for jb in range(nb):
                            w = 512 - rem if jb == nb - 1 else 512
                            dst = strip[qt][:, jb * 512:jb * 512 + w]
                            if jb > d_jb:
                                nc.scalar.copy(dst, mask_sb[:, 512:512 + w])
                                continue
                            ps_s = ps3.tile([128, 512], F32,
                                            name="ps_s", tag="ps_s")
                            for ec in range(2):
                                nc.tensor.matmul(
                                    ps_s[:, :w],
                                    qs[:, ec, qt * 128:(qt + 1) * 128],
                                    kp[:, ec, jb * 512:jb * 512 + w],
                                    start=(ec == 0),
                                    stop=(ec == 1),
                                )
                            if jb == d_jb:
                                off = 384 - 128 * (i % 4)
                                nc.vector.tensor_tensor(
                                    dst, ps_s[:, :w],
                                    mask_sb[:, off:off + w],
                                    mybir.AluOpType.add)
                            else:
                                nc.scalar.copy(dst, ps_s[:, :w])
                    return strip

                def consume(b, h, u, strip):
                    """softmax -> transpose P -> PV -> y^T drain."""
                    nb = (u + 2) // 2
                    nk = 2 * (u + 1)
                    Lp = nk * 128  # range PV reads (exp'd, masked -> 0)
                    pstrip = [
                        pstrips.tile([128, SEQ], BF16, name=f"pstrip{qt}",
                                     tag=f"pstrip{qt}")
                        for qt in range(2)
                    ]
                    for qt in range(2):
                        Lv = (2 * u + qt + 1) * 128  # causally valid cols
                        negmax = statp.tile([128, 1], F32, name="negmax",
                                            tag="negmax")
                        nc.vector.reduce_max(
                            negmax[:], strip[qt][:, :Lv],
                            axis=mybir.AxisListType.X, negate=True)
                        zsum = statp.tile([128, 1], F32, name="zsum",
                                          tag="zsum")
                        nc.scalar.activation(
                            pstrip[qt][:, :Lp], strip[qt][:, :Lp],
                            mybir.ActivationFunctionType.Exp,
                            bias=negmax[:], scale=1.0, accum_out=zsum[:])
                        rz = statp.tile([128, 1], F32, name="rz", tag="rz")
                        nc.vector.reciprocal(rz[:], zsum[:])
                        nc.vector.tensor_scalar_mul(
                            pstrip[qt][:, :Lp], pstrip[qt][:, :Lp], rz[:])
                    ps_y = [
                        ps3b.tile([128, 256], F32, name=f"ps_y{dh}",
                                  tag=f"ps_y{dh}")
                        for dh in range(2)
                    ]
                    for c in range(nk):
                        pt_sb = ptp.tile([128, 256], BF16, name="pt_sb",
                                         tag="pt_sb")
                        for qt in range(2):
                            ps_pt = ps3.tile([128, 128], BF16,
                                             name="ps_pt", tag="ps_pt")
                            nc.tensor.transpose(
                                ps_pt[:],
                                pstrip[qt][:, c * 128:(c + 1) * 128],
                                ident_sb[:])
                            nc.vector.tensor_copy(
                                pt_sb[:, qt * 128:(qt + 1) * 128], ps_pt[:])
                        g = b * (SEQ // 128) + c
                        for dh in range(2):
                            nc.tensor.matmul(
                                ps_y[dh][:],
                                v_all[:, g, h * 256 + dh * 128:
                                      h * 256 + (dh + 1) * 128],
                                pt_sb[:],
                                start=(c == 0),
                                stop=(c == nk - 1),
                            )
                    for dh in range(2):
                        nc.scalar.copy(
                            yt_t[(b, u // 2)][:, 2 * h + dh,
                                              (u % 2) * 256:
                                              (u % 2 + 1) * 256],
                            ps_y[dh][:])

                def proj_rs(b, sl):
                    """project one 512-token chunk of y^T, reduce-scatter."""
                    stg = b * nstg_b + sl
                    for ob in range(n_ob):
                        wpt = wpp.tile([128, 4, 512], F32R, name="wpt",
                                       tag="wpt")
                        for fc in range(4):
                            nc.sync.dma_start(
                                wpt[:, fc, :],
                                wp.ap()[fc, :, ob * 512:(ob + 1) * 512])
                        for st in range(4):
                            t0 = st * 128
                            ps_o = ps3o.tile([128, 512], F32, name="ps_o",
                                             tag="ps_o")
                            for fc in range(4):
                                nc.tensor.matmul(
                                    ps_o[:],
                                    yt_t[(b, sl)][:, fc, t0:t0 + 128],
                                    wpt[:, fc, :],
                                    start=(fc == 0),
                                    stop=(fc == 3),
                                )
                            ost = p4st.tile([128, 512], F32, name="ost",
                                            tag="ost")
                            nc.vector.tensor_copy(ost[:], ps_o[:])
                            nc.sync.dma_start(
                                partial_c[stg][t0:t0 + 128,
                                               ob * 512:(ob + 1) * 512],
                                ost[:])
                            if debug_dumps:
                                gt0 = stg * 512 + t0
                                nc.sync.dma_start(
                                    part_dump.ap()[gt0:gt0 + 128,
                                                   ob * 512:(ob + 1) * 512],
                                    ost[:])
                    nc.gpsimd.collective_compute(
                        "ReduceScatter",
                        mybir.AluOpType.add,
                        ins=[partial_c[stg].opt()],
                        outs=[rs_out_c[stg].opt()],
                        replica_groups=[list(range(N_CORES))],
                    )
                    nc.gpsimd.dma_start(out_ext.ap()[stg], rs_out_c[stg])

                stages = [(b, h, u)
                          for b in range(cfg.batch)
                          for h in range(HPC)
                          for u in range(cfg.supers)]
                def after_consume(pb, ph, pu):
                    if ph == HPC - 1 and pu % 2 == 1:
                        sl = pu // 2
                        if debug_dumps:
                            g0 = pb * SEQ + sl * 512
                            nc.sync.dma_start(
                                yt_dump.ap()[:, :, g0:g0 + 512],
                                yt_t[(pb, sl)][:].bitcast(F32))
                        proj_rs(pb, sl)

                prev = None
                for stage in stages:
                    strip = produce(*stage)
                    if prev is not None:
                        consume(*prev[0], prev[1])
                        after_consume(*prev[0])
                    prev = (stage, strip)
                consume(*prev[0], prev[1])
                after_consume(*prev[0])
            ytp_cm.__exit__(None, None, None)

    nc.finalize()
    return nc


def prep_inputs(cfg: Cfg, hidden_states, w_qkv, w_proj):
    """Shard + lay out the full inputs for each of the 8 cores."""
    seq, batch, e = hidden_states.shape
    assert (seq, batch, e) == (cfg.seq, cfg.batch, cfg.e)
    hs_t = np.ascontiguousarray(
        hidden_states.transpose(1, 0, 2).reshape(cfg.tok, e).T
    )  # [e, tok], tokens batch-major
    # [ntb, ech, 128, 256]
    hs5 = np.ascontiguousarray(
        hs_t.reshape(cfg.ech, 128, cfg.ntb, 256).transpose(2, 0, 1, 3)
    ).astype(np.float32)

    scale = math.sqrt(math.sqrt(KV_CHANNELS))
    w3 = w_qkv.reshape(HEADS, 3, HD, e)
    mask = np.full((128, 1024), 0.0, dtype=np.float32)
    cols = np.arange(1024)[None, :]
    rows = np.arange(128)[:, None]
    mask[cols > 384 + rows] = NEG
    import ml_dtypes
    ident = np.eye(128, dtype=ml_dtypes.bfloat16)

    in_maps = []
    for c in range(N_CORES):
        hsel = [2 * c, 2 * c + 1]
        w_q = (w3[hsel, 0] * scale).reshape(cfg.f_qk // 2, e)
        w_k = (w3[hsel, 1] * scale).reshape(cfg.f_qk // 2, e)
        w_v = w3[hsel, 2].reshape(cfg.f_v, e)
        wqk = np.concatenate([w_q, w_k], axis=0)  # [1024, e]
        wqk_t = np.ascontiguousarray(wqk.T.reshape(cfg.ech, 128, cfg.f_qk))
        wv_t = np.ascontiguousarray(w_v.T.reshape(cfg.ech, 128, cfg.f_v))
        wp_c = w_proj[:, c * cfg.f_v:(c + 1) * cfg.f_v]  # [OUT, 512]
        wp_t = np.ascontiguousarray(wp_c.T.reshape(4, 128, cfg.out))
        in_maps.append({
            "hs5": hs5,
            "wqk": wqk_t.astype(np.float32),
            "wv": wv_t.astype(np.float32),
            "wp": wp_t.astype(np.float32),
            "maskm": mask,
            "ident": ident,
        })
    return in_maps


def assemble_output(cfg: Cfg, results):
    """Gather per-core ReduceScatter shards into the full [seq, b, out]."""
    rows = 512 // N_CORES
    full = np.empty((cfg.tok, cfg.out), dtype=np.float32)
    for stg in range(cfg.nstg):
        for r in range(N_CORES):
            t0 = stg * 512 + r * rows
            full[t0:t0 + rows] = results[r]["out_ext"][stg]
    return np.ascontiguousarray(
        full.reshape(cfg.batch, cfg.seq, cfg.out).transpose(1, 0, 2))


_NC_CACHE = {}


def run(cfg: Cfg, hidden_states, w_qkv, w_proj, trace=False):
    key = (cfg.seq, cfg.e, cfg.out)
    if key not in _NC_CACHE:
        _NC_CACHE[key] = build_kernel(cfg)
    nc = _NC_CACHE[key]
    in_maps = prep_inputs(cfg, hidden_states, w_qkv, w_proj)
    res = bass_utils.run_bass_kernel_spmd(
        nc, in_maps, core_ids=list(range(N_CORES)), trace=trace)
    return assemble_output(cfg, res.results), res


def kernel(hidden_states, attention_mask, w_qkv, w_proj):
    cfg = Cfg()
    out, _ = run(cfg, np.asarray(hidden_states, dtype=np.float32),
                 np.asarray(w_qkv, dtype=np.float32),
                 np.asarray(w_proj, dtype=np.float32))
    return out

